# revision 22
# baseline (speedup 1.0000x reference)
"""LoRA layer kernel for Trainium2 (8 NeuronCores, data-parallel).

Computes out = SCALING * (x @ A^T) @ B^T for x [4, 8192, 1024],
lora_A [4, 1024], lora_B [1024, 4], SCALING = 0.25.

Strategy (per core, shard = 4096 rows x 1024 features):
  - x rows are sharded across the 8 cores; A/B replicated (host pre-arranged).
  - Per 512-row group: DMA x in natural layout, transpose 128x128 chunks on
    the PE (fp32r transpose mode) into PSUM, evacuate to SBUF with the DVE,
    rank-4 matmuls (fp32r, N=512) for h^T = A x^T, then out = h @ (0.25 B^T)
    with rows back on partitions so the store is contiguous; ScalarE
    evacuates the output PSUM banks; one 2 MiB DMA store per group.
"""

import sys

for _p in (
    "/root/.axon_site",
    "/root/.axon_site/_ro/trn_rl_repo",
    "/root/.axon_site/_ro/pypackages",
):
    if _p not in sys.path:
        sys.path.insert(0, _p)

from contextlib import ExitStack

import numpy as np

N_CORES = 8
D_IN = 1024
D_OUT = 1024
RANK = 4
ROWS_TOTAL = 4 * 8192
ROWS_PER_CORE = ROWS_TOTAL // N_CORES  # 4096
SCALING = 1.0 / RANK

P = 128          # partitions
GROUP_ROWS = 512  # rows processed per pipeline iteration (4 subtiles of 128)
N_CHUNKS = D_IN // P  # 8 feature chunks


def emit_lora(tc, x_ap, at_ap, bt_ap, id_ap, out_ap, rows):
    """Emit the LoRA kernel IR for one core's shard of `rows` rows.

    x_ap  : DRAM [rows, D_IN]  f32 (declared f32r; raw f32 bits)
    at_ap : DRAM [P, N_CHUNKS, RANK] f32r, at[p, c, r] = A[r, c*P + p]
    bt_ap : DRAM [RANK, D_OUT] f32r, bt[r, o] = SCALING * B[o, r]
    id_ap : DRAM [P, P] f32r identity (for PE transpose)
    out_ap: DRAM [rows, D_OUT] f32
    """
    import concourse.mybir as mybir

    nc = tc.nc
    f32 = mybir.dt.float32
    f32r = mybir.dt.float32r
    ctx = tc._ctx  # ExitStack owned by caller

    n_groups = rows // GROUP_ROWS
    J = GROUP_ROWS // P  # 4 row subtiles per group
    n_ochunks = D_OUT // 512  # 2 output column chunks of 512

    consts = ctx.enter_context(tc.tile_pool(name="consts", bufs=1))
    xpool = ctx.enter_context(tc.tile_pool(name="xin", bufs=5))
    xtpool = ctx.enter_context(tc.tile_pool(name="xt", bufs=6))
    htpool = ctx.enter_context(tc.tile_pool(name="ht", bufs=2))
    opool = ctx.enter_context(tc.tile_pool(name="osb", bufs=3))
    ps_xt = ctx.enter_context(tc.tile_pool(name="ps_xt", bufs=3, space="PSUM"))
    ps_ht = ctx.enter_context(tc.tile_pool(name="ps_ht", bufs=2, space="PSUM"))
    ps_o = ctx.enter_context(tc.tile_pool(name="ps_o", bufs=3, space="PSUM"))

    ident = consts.tile([P, P], f32r)
    nc.gpsimd.dma_start(ident[:], id_ap[:])
    at_sb = consts.tile([P, N_CHUNKS, RANK], f32r)
    nc.gpsimd.dma_start(at_sb[:], at_ap[:])
    bt_sb = consts.tile([RANK, D_OUT], f32r)
    nc.gpsimd.dma_start(bt_sb[:], bt_ap[:])

    # rows -> partitions: row = n*P + p
    x_pnm = x_ap.rearrange("(n p) m -> p n m", p=P)
    o_pnm = out_ap.rearrange("(n p) m -> p n m", p=P)

    for g in range(n_groups):
        x_sb = xpool.tile([P, J, D_IN], f32r)
        for j in range(J):
            nc.sync.dma_start(x_sb[:, j, :], x_pnm[:, g * J + j, :])

        ht_ps = ps_ht.tile([RANK, GROUP_ROWS], f32)
        for c in range(N_CHUNKS):
            # Transpose the 4 row-subtiles of feature chunk c into one PSUM
            # bank: xt_ps[p=feat, j, m=row] = x[row, feat]. One accumulation
            # group per bank (start on first write, stop on last).
            xt_ps = ps_xt.tile([P, J, P], f32r)
            for j in range(J):
                nc.tensor.matmul(
                    xt_ps[:, j, :],
                    lhsT=x_sb[:, j, c * P : (c + 1) * P],
                    rhs=ident[:],
                    is_transpose=True,
                    start=(j == 0),
                    stop=(j == J - 1),
                )
            xt_sb = xtpool.tile([P, J, P], f32r)
            nc.vector.tensor_copy(xt_sb[:], xt_ps[:])
            # h^T[r, m] += sum_f A^T[cP+f, r] * xT[f, m]
            nc.tensor.matmul(
                ht_ps[:],
                lhsT=at_sb[:, c, :],
                rhs=xt_sb[:],
                start=(c == 0),
                stop=(c == N_CHUNKS - 1),
            )

        ht_sb = htpool.tile([RANK, GROUP_ROWS], f32r)
        nc.vector.tensor_copy(ht_sb[:], ht_ps[:])

        o_sb = opool.tile([P, J, D_OUT], f32)
        for j in range(J):
            for o2 in range(n_ochunks):
                o_ps = ps_o.tile([P, 512], f32)
                # out[m, o] = sum_r h^T[r, m] * bt[r, o]
                nc.tensor.matmul(
                    o_ps[:],
                    lhsT=ht_sb[:, j * P : (j + 1) * P],
                    rhs=bt_sb[:, o2 * 512 : (o2 + 1) * 512],
                    start=True,
                    stop=True,
                )
                nc.scalar.copy(o_sb[:, j, o2 * 512 : (o2 + 1) * 512], o_ps[:])
            nc.scalar.dma_start(o_pnm[:, g * J + j, :], o_sb[:, j, :])


def build_nc(rows=ROWS_PER_CORE):
    import concourse.mybir as mybir
    import concourse.tile as tile
    from concourse import bacc

    f32 = mybir.dt.float32
    f32r = mybir.dt.float32r
    nc = bacc.Bacc("TRN2", target_bir_lowering=False, debug=False)
    x_d = nc.dram_tensor("x", [rows, D_IN], f32r, kind="ExternalInput").ap()
    at_d = nc.dram_tensor("at", [P, N_CHUNKS, RANK], f32r, kind="ExternalInput").ap()
    bt_d = nc.dram_tensor("bt", [RANK, D_OUT], f32r, kind="ExternalInput").ap()
    id_d = nc.dram_tensor("ident", [P, P], f32r, kind="ExternalInput").ap()
    out_d = nc.dram_tensor("out", [rows, D_OUT], f32, kind="ExternalOutput").ap()

    with tile.TileContext(nc) as tc:
        with ExitStack() as ctx:
            tc._ctx = ctx
            emit_lora(tc, x_d, at_d, bt_d, id_d, out_d, rows)
    nc.compile()
    return nc


def round_tf32(a):
    """Round f32 to tfloat32 (10-bit mantissa), round-to-nearest-even."""
    u = np.ascontiguousarray(a, dtype=np.float32).view(np.uint32)
    r = (u + 0x0FFF + ((u >> 13) & 1)) & np.uint32(0xFFFFE000)
    return r.view(np.float32)


def host_prep(lora_A, lora_B):
    # at[p, c, r] = A[r, c*P + p]
    at = np.ascontiguousarray(
        lora_A.T.reshape(N_CHUNKS, P, RANK).transpose(1, 0, 2), dtype=np.float32
    )
    bt = np.ascontiguousarray(lora_B.T * SCALING, dtype=np.float32)
    return round_tf32(at), round_tf32(bt)


_NC_CACHE = {}


def kernel(x, lora_A, lora_B):
    from concourse.bass_utils import run_bass_kernel_spmd

    if "nc" not in _NC_CACHE:
        _NC_CACHE["nc"] = build_nc(ROWS_PER_CORE)
    nc = _NC_CACHE["nc"]

    x2 = np.ascontiguousarray(x, dtype=np.float32).reshape(ROWS_TOTAL, D_IN)
    at, bt = host_prep(np.asarray(lora_A), np.asarray(lora_B))
    ident = np.eye(P, dtype=np.float32)
    shards = x2.reshape(N_CORES, ROWS_PER_CORE, D_IN)
    in_maps = [
        {"x": np.ascontiguousarray(shards[i]), "at": at, "bt": bt, "ident": ident}
        for i in range(N_CORES)
    ]
    res = run_bass_kernel_spmd(nc, in_maps, core_ids=list(range(N_CORES)))
    out = np.concatenate([res.results[i]["out"] for i in range(N_CORES)], axis=0)
    return out.reshape(4, 8192, D_OUT)


# revision 23
# speedup vs baseline: 1.0735x; 1.0735x over previous
"""LoRA layer kernel for Trainium2 (8 NeuronCores, data-parallel).

Computes out = SCALING * (x @ A^T) @ B^T for x [4, 8192, 1024],
lora_A [4, 1024], lora_B [1024, 4], SCALING = 0.25.

Strategy (per core, shard = 4096 rows x 1024 features):
  - x rows are sharded across the 8 cores; A/B replicated (host pre-arranged).
  - Per 512-row group: DMA x in natural layout, transpose 128x128 chunks on
    the PE (fp32r transpose mode) into PSUM, evacuate to SBUF with the DVE,
    rank-4 matmuls (fp32r, N=512) for h^T = A x^T, then out = h @ (0.25 B^T)
    with rows back on partitions so the store is contiguous; ScalarE
    evacuates the output PSUM banks; one 2 MiB DMA store per group.
"""

import sys

for _p in (
    "/root/.axon_site",
    "/root/.axon_site/_ro/trn_rl_repo",
    "/root/.axon_site/_ro/pypackages",
):
    if _p not in sys.path:
        sys.path.insert(0, _p)

from contextlib import ExitStack

import numpy as np

N_CORES = 8
D_IN = 1024
D_OUT = 1024
RANK = 4
ROWS_TOTAL = 4 * 8192
ROWS_PER_CORE = ROWS_TOTAL // N_CORES  # 4096
SCALING = 1.0 / RANK

P = 128          # partitions
GROUP_ROWS = 512  # rows processed per pipeline iteration (4 subtiles of 128)
N_CHUNKS = D_IN // P  # 8 feature chunks


def emit_lora(tc, x_ap, at_ap, bt_ap, id_ap, out_ap, rows):
    """Emit the LoRA kernel IR for one core's shard of `rows` rows.

    x_ap  : DRAM [rows, D_IN]  f32 (declared f32r; raw f32 bits)
    at_ap : DRAM [P, N_CHUNKS, RANK] f32r, at[p, c, r] = A[r, c*P + p]
    bt_ap : DRAM [RANK, D_OUT] f32r, bt[r, o] = SCALING * B[o, r]
    id_ap : DRAM [P, P] f32r identity (for PE transpose)
    out_ap: DRAM [rows, D_OUT] f32
    """
    import concourse.mybir as mybir

    nc = tc.nc
    f32 = mybir.dt.float32
    f32r = mybir.dt.float32r
    ctx = tc._ctx  # ExitStack owned by caller

    n_groups = rows // GROUP_ROWS
    J = GROUP_ROWS // P  # 4 row subtiles per group
    n_ochunks = D_OUT // 512  # 2 output column chunks of 512

    consts = ctx.enter_context(tc.tile_pool(name="consts", bufs=1))
    xpool = ctx.enter_context(tc.tile_pool(name="xin", bufs=5))
    xtpool = ctx.enter_context(tc.tile_pool(name="xt", bufs=6))
    htpool = ctx.enter_context(tc.tile_pool(name="ht", bufs=2))
    opool = ctx.enter_context(tc.tile_pool(name="osb", bufs=3))
    ps_xt = ctx.enter_context(tc.tile_pool(name="ps_xt", bufs=3, space="PSUM"))
    ps_ht = ctx.enter_context(tc.tile_pool(name="ps_ht", bufs=2, space="PSUM"))
    ps_o = ctx.enter_context(tc.tile_pool(name="ps_o", bufs=3, space="PSUM"))

    ident = consts.tile([P, P], f32r)
    nc.gpsimd.dma_start(ident[:], id_ap[:])
    at_sb = consts.tile([P, N_CHUNKS, RANK], f32r)
    nc.gpsimd.dma_start(at_sb[:], at_ap[:])
    bt_sb = consts.tile([RANK, D_OUT], f32r)
    nc.gpsimd.dma_start(bt_sb[:], bt_ap[:])

    # rows -> partitions: row = n*P + p
    x_pnm = x_ap.rearrange("(n p) m -> p n m", p=P)
    o_pnm = out_ap.rearrange("(n p) m -> p n m", p=P)

    for g in range(n_groups):
        x_sb = xpool.tile([P, J, D_IN], f32r)
        for j in range(J):
            nc.sync.dma_start(x_sb[:, j, :], x_pnm[:, g * J + j, :])

        ht_ps = ps_ht.tile([RANK, GROUP_ROWS], f32)
        for c in range(N_CHUNKS):
            # Transpose the 4 row-subtiles of feature chunk c into one PSUM
            # bank: xt_ps[p=feat, j, m=row] = x[row, feat]. One accumulation
            # group per bank (start on first write, stop on last).
            xt_ps = ps_xt.tile([P, J, P], f32r)
            for j in range(J):
                nc.tensor.matmul(
                    xt_ps[:, j, :],
                    lhsT=x_sb[:, j, c * P : (c + 1) * P],
                    rhs=ident[:],
                    is_transpose=True,
                    start=(j == 0),
                    stop=(j == J - 1),
                )
            xt_sb = xtpool.tile([P, J, P], f32r)
            nc.vector.tensor_copy(xt_sb[:], xt_ps[:])
            # h^T[r, m] += sum_f A^T[cP+f, r] * xT[f, m]
            nc.tensor.matmul(
                ht_ps[:],
                lhsT=at_sb[:, c, :],
                rhs=xt_sb[:],
                start=(c == 0),
                stop=(c == N_CHUNKS - 1),
            )

        ht_sb = htpool.tile([RANK, GROUP_ROWS], f32r)
        nc.vector.tensor_copy(ht_sb[:], ht_ps[:])

        o_sb = opool.tile([P, J, D_OUT], f32)
        for j in range(J):
            for o2 in range(n_ochunks):
                o_ps = ps_o.tile([P, 512], f32)
                # out[m, o] = sum_r h^T[r, m] * bt[r, o]
                nc.tensor.matmul(
                    o_ps[:],
                    lhsT=ht_sb[:, j * P : (j + 1) * P],
                    rhs=bt_sb[:, o2 * 512 : (o2 + 1) * 512],
                    start=True,
                    stop=True,
                )
                nc.scalar.copy(o_sb[:, j, o2 * 512 : (o2 + 1) * 512], o_ps[:])
            nc.sync.dma_start(o_pnm[:, g * J + j, :], o_sb[:, j, :])


def build_nc(rows=ROWS_PER_CORE):
    import concourse.mybir as mybir
    import concourse.tile as tile
    from concourse import bacc

    f32 = mybir.dt.float32
    f32r = mybir.dt.float32r
    nc = bacc.Bacc("TRN2", target_bir_lowering=False, debug=False)
    x_d = nc.dram_tensor("x", [rows, D_IN], f32r, kind="ExternalInput").ap()
    at_d = nc.dram_tensor("at", [P, N_CHUNKS, RANK], f32r, kind="ExternalInput").ap()
    bt_d = nc.dram_tensor("bt", [RANK, D_OUT], f32r, kind="ExternalInput").ap()
    id_d = nc.dram_tensor("ident", [P, P], f32r, kind="ExternalInput").ap()
    out_d = nc.dram_tensor("out", [rows, D_OUT], f32, kind="ExternalOutput").ap()

    with tile.TileContext(nc) as tc:
        with ExitStack() as ctx:
            tc._ctx = ctx
            emit_lora(tc, x_d, at_d, bt_d, id_d, out_d, rows)
    nc.compile()
    return nc


def round_tf32(a):
    """Round f32 to tfloat32 (10-bit mantissa), round-to-nearest-even."""
    u = np.ascontiguousarray(a, dtype=np.float32).view(np.uint32)
    r = (u + 0x0FFF + ((u >> 13) & 1)) & np.uint32(0xFFFFE000)
    return r.view(np.float32)


def host_prep(lora_A, lora_B):
    # at[p, c, r] = A[r, c*P + p]
    at = np.ascontiguousarray(
        lora_A.T.reshape(N_CHUNKS, P, RANK).transpose(1, 0, 2), dtype=np.float32
    )
    bt = np.ascontiguousarray(lora_B.T * SCALING, dtype=np.float32)
    return round_tf32(at), round_tf32(bt)


_NC_CACHE = {}


def kernel(x, lora_A, lora_B):
    from concourse.bass_utils import run_bass_kernel_spmd

    if "nc" not in _NC_CACHE:
        _NC_CACHE["nc"] = build_nc(ROWS_PER_CORE)
    nc = _NC_CACHE["nc"]

    x2 = np.ascontiguousarray(x, dtype=np.float32).reshape(ROWS_TOTAL, D_IN)
    at, bt = host_prep(np.asarray(lora_A), np.asarray(lora_B))
    ident = np.eye(P, dtype=np.float32)
    shards = x2.reshape(N_CORES, ROWS_PER_CORE, D_IN)
    in_maps = [
        {"x": np.ascontiguousarray(shards[i]), "at": at, "bt": bt, "ident": ident}
        for i in range(N_CORES)
    ]
    res = run_bass_kernel_spmd(nc, in_maps, core_ids=list(range(N_CORES)))
    out = np.concatenate([res.results[i]["out"] for i in range(N_CORES)], axis=0)
    return out.reshape(4, 8192, D_OUT)


# revision 25
# speedup vs baseline: 1.1736x; 1.0932x over previous
"""LoRA layer kernel for Trainium2 (8 NeuronCores, data-parallel).

Computes out = SCALING * (x @ A^T) @ B^T for x [4, 8192, 1024],
lora_A [4, 1024], lora_B [1024, 4], SCALING = 0.25.

Strategy (per core, shard = 4096 rows x 1024 features):
  - x rows are sharded across the 8 cores; A/B replicated (host pre-arranged).
  - Per 512-row group: DMA x in natural layout, transpose 128x128 chunks on
    the PE (fp32r transpose mode) into PSUM, evacuate to SBUF with the DVE,
    rank-4 matmuls (fp32r, N=512) for h^T = A x^T, then out = h @ (0.25 B^T)
    with rows back on partitions so the store is contiguous; ScalarE
    evacuates the output PSUM banks; one 2 MiB DMA store per group.
"""

import sys

for _p in (
    "/root/.axon_site",
    "/root/.axon_site/_ro/trn_rl_repo",
    "/root/.axon_site/_ro/pypackages",
):
    if _p not in sys.path:
        sys.path.insert(0, _p)

from contextlib import ExitStack

import numpy as np

N_CORES = 8
D_IN = 1024
D_OUT = 1024
RANK = 4
ROWS_TOTAL = 4 * 8192
ROWS_PER_CORE = ROWS_TOTAL // N_CORES  # 4096
SCALING = 1.0 / RANK

P = 128          # partitions
GROUP_ROWS = 512  # rows processed per pipeline iteration (4 subtiles of 128)
N_CHUNKS = D_IN // P  # 8 feature chunks


def emit_lora(tc, x_ap, at_ap, bt_ap, id_ap, out_ap, rows):
    """Emit the LoRA kernel IR for one core's shard of `rows` rows.

    x_ap  : DRAM [rows, D_IN]  f32 (declared f32r; raw f32 bits)
    at_ap : DRAM [P, N_CHUNKS, RANK] f32r, at[p, c, r] = A[r, c*P + p]
    bt_ap : DRAM [RANK, D_OUT] f32r, bt[r, o] = SCALING * B[o, r]
    id_ap : DRAM [P, P] f32r identity (for PE transpose)
    out_ap: DRAM [rows, D_OUT] f32
    """
    import concourse.mybir as mybir

    nc = tc.nc
    f32 = mybir.dt.float32
    f32r = mybir.dt.float32r
    ctx = tc._ctx  # ExitStack owned by caller

    n_groups = rows // GROUP_ROWS
    J = GROUP_ROWS // P  # 4 row subtiles per group
    n_ochunks = D_OUT // 512  # 2 output column chunks of 512

    consts = ctx.enter_context(tc.tile_pool(name="consts", bufs=1))
    xpool = ctx.enter_context(tc.tile_pool(name="xin", bufs=5))
    xtpool = ctx.enter_context(tc.tile_pool(name="xt", bufs=6))
    htpool = ctx.enter_context(tc.tile_pool(name="ht", bufs=2))
    opool = ctx.enter_context(tc.tile_pool(name="osb", bufs=3))
    ps_xt = ctx.enter_context(tc.tile_pool(name="ps_xt", bufs=3, space="PSUM"))
    ps_ht = ctx.enter_context(tc.tile_pool(name="ps_ht", bufs=2, space="PSUM"))
    ps_o = ctx.enter_context(tc.tile_pool(name="ps_o", bufs=3, space="PSUM"))

    ident = consts.tile([P, P], f32r)
    nc.sync.dma_start(ident[:], id_ap[:])
    at_sb = consts.tile([P, N_CHUNKS, RANK], f32r)
    nc.sync.dma_start(at_sb[:], at_ap[:])
    bt_sb = consts.tile([RANK, D_OUT], f32r)
    nc.sync.dma_start(bt_sb[:], bt_ap[:])

    # rows -> partitions: row = n*P + p
    x_pnm = x_ap.rearrange("(n p) m -> p n m", p=P)
    o_pnm = out_ap.rearrange("(n p) m -> p n m", p=P)

    for g in range(n_groups):
        x_sb = xpool.tile([P, J, D_IN], f32r)
        for j in range(J):
            nc.sync.dma_start(x_sb[:, j, :], x_pnm[:, g * J + j, :])

        ht_ps = ps_ht.tile([RANK, GROUP_ROWS], f32)
        for c in range(N_CHUNKS):
            # Transpose the 4 row-subtiles of feature chunk c into one PSUM
            # bank: xt_ps[p=feat, j, m=row] = x[row, feat]. One accumulation
            # group per bank (start on first write, stop on last).
            xt_ps = ps_xt.tile([P, J, P], f32r)
            for j in range(J):
                nc.tensor.matmul(
                    xt_ps[:, j, :],
                    lhsT=x_sb[:, j, c * P : (c + 1) * P],
                    rhs=ident[:],
                    is_transpose=True,
                    start=(j == 0),
                    stop=(j == J - 1),
                )
            xt_sb = xtpool.tile([P, J, P], f32r)
            nc.vector.tensor_copy(xt_sb[:], xt_ps[:])
            # h^T[r, m] += sum_f A^T[cP+f, r] * xT[f, m]
            nc.tensor.matmul(
                ht_ps[:],
                lhsT=at_sb[:, c, :],
                rhs=xt_sb[:],
                start=(c == 0),
                stop=(c == N_CHUNKS - 1),
            )

        ht_sb = htpool.tile([RANK, GROUP_ROWS], f32r)
        nc.vector.tensor_copy(ht_sb[:], ht_ps[:])

        o_sb = opool.tile([P, J, D_OUT], f32)
        for j in range(J):
            for o2 in range(n_ochunks):
                o_ps = ps_o.tile([P, 512], f32)
                # out[m, o] = sum_r h^T[r, m] * bt[r, o]
                nc.tensor.matmul(
                    o_ps[:],
                    lhsT=ht_sb[:, j * P : (j + 1) * P],
                    rhs=bt_sb[:, o2 * 512 : (o2 + 1) * 512],
                    start=True,
                    stop=True,
                )
                nc.scalar.copy(o_sb[:, j, o2 * 512 : (o2 + 1) * 512], o_ps[:])

        # Stores ride the SWDGE (gpsimd) ring so a store waiting on its copy
        # never head-of-line-blocks the HWDGE load ring.
        nc.gpsimd.dma_start(o_pnm[:, g * J : (g + 1) * J, :], o_sb[:])


def build_nc(rows=ROWS_PER_CORE):
    import concourse.mybir as mybir
    import concourse.tile as tile
    from concourse import bacc

    f32 = mybir.dt.float32
    f32r = mybir.dt.float32r
    nc = bacc.Bacc("TRN2", target_bir_lowering=False, debug=False)
    x_d = nc.dram_tensor("x", [rows, D_IN], f32r, kind="ExternalInput").ap()
    at_d = nc.dram_tensor("at", [P, N_CHUNKS, RANK], f32r, kind="ExternalInput").ap()
    bt_d = nc.dram_tensor("bt", [RANK, D_OUT], f32r, kind="ExternalInput").ap()
    id_d = nc.dram_tensor("ident", [P, P], f32r, kind="ExternalInput").ap()
    out_d = nc.dram_tensor("out", [rows, D_OUT], f32, kind="ExternalOutput").ap()

    with tile.TileContext(nc) as tc:
        with ExitStack() as ctx:
            tc._ctx = ctx
            emit_lora(tc, x_d, at_d, bt_d, id_d, out_d, rows)
    nc.compile()
    return nc


def round_tf32(a):
    """Round f32 to tfloat32 (10-bit mantissa), round-to-nearest-even."""
    u = np.ascontiguousarray(a, dtype=np.float32).view(np.uint32)
    r = (u + 0x0FFF + ((u >> 13) & 1)) & np.uint32(0xFFFFE000)
    return r.view(np.float32)


def host_prep(lora_A, lora_B):
    # at[p, c, r] = A[r, c*P + p]
    at = np.ascontiguousarray(
        lora_A.T.reshape(N_CHUNKS, P, RANK).transpose(1, 0, 2), dtype=np.float32
    )
    bt = np.ascontiguousarray(lora_B.T * SCALING, dtype=np.float32)
    return round_tf32(at), round_tf32(bt)


_NC_CACHE = {}


def kernel(x, lora_A, lora_B):
    from concourse.bass_utils import run_bass_kernel_spmd

    if "nc" not in _NC_CACHE:
        _NC_CACHE["nc"] = build_nc(ROWS_PER_CORE)
    nc = _NC_CACHE["nc"]

    x2 = np.ascontiguousarray(x, dtype=np.float32).reshape(ROWS_TOTAL, D_IN)
    at, bt = host_prep(np.asarray(lora_A), np.asarray(lora_B))
    ident = np.eye(P, dtype=np.float32)
    shards = x2.reshape(N_CORES, ROWS_PER_CORE, D_IN)
    in_maps = [
        {"x": np.ascontiguousarray(shards[i]), "at": at, "bt": bt, "ident": ident}
        for i in range(N_CORES)
    ]
    res = run_bass_kernel_spmd(nc, in_maps, core_ids=list(range(N_CORES)))
    out = np.concatenate([res.results[i]["out"] for i in range(N_CORES)], axis=0)
    return out.reshape(4, 8192, D_OUT)
